# revision 35
# baseline (speedup 1.0000x reference)
"""Causal multi-head self-attention (B=2, S=2048, D=1024, H=16) on 8 trn2 cores.

Sharding: tensor-parallel over heads. Each core owns 2 heads (a 128-wide
slice of the QKV output dim / o_proj input dim), computes QKV projection,
causal attention and its partial output projection; the host sums the 8
partial outputs (the TP unshard step).

Device kernel layout (per core, SPMD):
  - Everything is computed in "transposed" orientation so no transposes of
    attention probabilities are ever needed:
      qT, kT  : [dloc=128, T]   (feature-major)  qT pre-scaled by 1/sqrt(dk)
      scoresT : [k_chunk=128, q_tile=512] = kT_chunk.T-contract @ qT
      PT      : exp(scoresT)  (block-causal; diagonal 128x128 blocks masked
                by a DVE multiply with an uploaded 0/1 mask)
      oT      : accum over k chunks of V_aug.T @ PT, where V_aug carries a
                64-wide ones block so PSUM rows 64:128 hold 64 replicated
                copies of the softmax denominator (free PE broadcast).
      out     : [tok, 1024] partial = oT(normalized) as lhsT @ woT
  - Unmasked chunk pairs run the AV matmul as one fp8e4m3 DoubleRow pass
    (2 k-chunks per 512-column stream); masked (diagonal) chunks stay bf16
    so short causal rows keep bf16 accuracy.
  - Software pipeline: batch 0's QKV projection runs first (PE-bound, ACT
    idle); batch 0's attention (ACT/exp-paced) then runs with batch 1's
    QKV chains interleaved as PE filler between attention pair-blocks;
    batch 1's attention runs with the deferred output projections as
    filler.  This keeps the PE issue queue non-blocking through the
    exp-paced stretches instead of phase-serial QKV-then-attention.
  - One PSUM pool set for the whole kernel: pss 2x[128,2,512] score pairs
    (4 banks), pso 2x[128,512] AV accumulators (2 banks), psx 2x[128,512]
    shared ring for warmup/QKV chains/V transposes/o_proj (2 banks).
  - Normalization: po staged to partition-base-0 SBUF tiles, then
    reciprocal_approx_fast + one multiply into oT.  NOTE (HW-proven):
    reciprocal_approx_fast must read a base-0 tile -- feeding it a
    partition-offset slice (po[64:128] or an SBUF slice at offset 64)
    produces garbage on hardware while CoreSim accepts it.

dtype modes: "bf16" (default/fastest, rel err ~5e-3 vs the 2e-2 gate),
"f32r" (tf32-ish matmuls, ~2e-4), "f32" (exact, PE at 1/4 rate).
"""

import numpy as np
import ml_dtypes
from collections import deque

import concourse.bass as bass
import concourse.mybir as mybir
import concourse.tile as tile
from concourse import bacc
from concourse.bass_utils import run_bass_kernel_spmd

# Problem config (hardcoded; harness contract).
B, S, D, NH = 2, 2048, 1024, 16
NCORES = 8
DK = D // NH                # 64
H_LOC = NH // NCORES        # 2 heads per core
DLOC = H_LOC * DK           # 128
T = B * S                   # 4096

MM_MODE = "bf16"            # "f32" | "f32r" | "bf16"

F32 = mybir.dt.float32
BF16 = mybir.dt.bfloat16
F32R = mybir.dt.float32r


def build_program(mm_mode=MM_MODE, b=B, s=S, d=D, nh=NH, ncores=NCORES,
                  debug_dump=False):
    dk = d // nh
    h_loc = nh // ncores
    dloc = h_loc * dk
    t_all = b * s

    QT_ = 512                     # q tile size (matmul moving free dim)
    KC = 128                      # k chunk size (partition dim)
    assert s % QT_ == 0 and d % 128 == 0 and dloc == 128 and dk == 64

    n_qt = s // QT_               # q tiles per batch
    n_groups = t_all // QT_       # token groups for QKV projection
    n_kd = d // 128               # contraction chunks over d_model
    n_tc = t_all // KC            # token chunks (for V layout)

    st = {"bf16": BF16, "f32r": F32R, "f32": F32}[mm_mode]
    avdt = mybir.dt.float8e4 if mm_mode == "bf16" else st
    av_fp8 = avdt == mybir.dt.float8e4
    exp_bias = -1.0 if av_fp8 else 0.0
    vtdt = BF16 if mm_mode == "bf16" else F32
    odt = BF16 if mm_mode == "bf16" else F32

    nc = bacc.Bacc("TRN2", target_bir_lowering=False, debug=False,
                   enable_asserts=False)

    # xT / wqkvT are host-pre-swizzled into the exact SBUF tile layouts so
    # every DMA is fully contiguous (strided patterns measured ~4x slower)
    xT = nc.dram_tensor("xT", [n_groups, 128, n_kd, QT_], st,
                        kind="ExternalInput")
    wqkvT = nc.dram_tensor("wqkvT", [128, n_kd, 3 * dloc], st,
                           kind="ExternalInput")
    woT = nc.dram_tensor("woT", [dloc, d], st, kind="ExternalInput")
    dmask = nc.dram_tensor("dmask", [128, 128], st, kind="ExternalInput")
    ident = nc.dram_tensor("ident", [128, 128], vtdt, kind="ExternalInput")
    part = nc.dram_tensor("part", [t_all, d], odt, kind="ExternalOutput")
    dbg = {}
    if debug_dump:
        for nm, shp, dt_ in [("qT", [128, t_all], st),
                             ("kT", [128, t_all], st),
                             ("va0", [128, t_all // 128, 128], st),
                             ("oT", [128, t_all], st),
                             ("wo", [dloc, d], st)]:
            dbg[nm] = nc.dram_tensor(f"dbg_{nm}", shp, dt_,
                                     kind="ExternalOutput")

    with tile.TileContext(nc) as tc:
        with (
            tc.tile_pool(name="persist", bufs=1) as pp,
            tc.tile_pool(name="small", bufs=4) as sp,
        ):
            # ---- persistent SBUF tensors ----
            wo_sb = pp.tile([dloc, d], st, tag="wo")
            dm_sb = pp.tile([128, 128], st, tag="dm")
            id_sb = pp.tile([128, 128], vtdt, tag="id")

            # scratch for PE warm-up matmuls (HAM clock-gate release)
            warm_sb = pp.tile([128, 512], st, tag="warm")
            nc.vector.memset(warm_sb[:], 0.0)

            qT_sb = pp.tile([128, t_all], st, tag="qT")
            kT_sb = pp.tile([128, t_all], st, tag="kT")
            va_sb = [pp.tile([128, n_tc, 128], st, tag=f"va{h}",
                             name=f"va{h}") for h in range(h_loc)]
            va8_sb = [pp.tile([128, n_tc, 128], avdt, tag=f"va8{h}",
                              name=f"va8{h}") for h in range(h_loc)
                      ] if av_fp8 else None
            ones_c = pp.tile([128, 1], F32, tag="ones_c")
            nc.vector.memset(ones_c[:], 1.0)
            ebias_c = pp.tile([128, 1], F32, tag="ebias_c")
            nc.vector.memset(ebias_c[:], exp_bias)
            for h in range(h_loc):
                nc.vector.tensor_copy(
                    va_sb[h][:, :, dk:128],
                    ones_c[:, :, None].broadcast_to((128, n_tc, 128 - dk)))
                if av_fp8:
                    nc.vector.tensor_copy(
                        va8_sb[h][:, :, dk:128],
                        ones_c[:, :, None].broadcast_to(
                            (128, n_tc, 128 - dk)))
            oT_sb = pp.tile([128, t_all], st, tag="oT")

            scale_q = 1.0 / float(np.sqrt(dk))

            with (
                tc.tile_pool(name="qkvw", bufs=1) as wp,
                tc.tile_pool(name="xg", bufs=8) as xp,
                tc.tile_pool(name="vtmp", bufs=1) as vp,
                tc.tile_pool(name="pt", bufs=10) as ptp,
                tc.tile_pool(name="onb", bufs=2) as onp,
                tc.tile_pool(name="ob", bufs=4) as obp,
                tc.tile_pool(name="ps_s", bufs=2, space="PSUM") as pss,
                tc.tile_pool(name="ps_o", bufs=1, space="PSUM") as pso,
                tc.tile_pool(name="ps_x", bufs=2, space="PSUM") as psx,
            ):
                wq_sb = wp.tile([128, n_kd, 3 * dloc], st, tag="wq")
                vT_tmp = vp.tile([128, t_all], vtdt, tag="vtmp")

                # PE warm-up: dummy matmuls with no DMA dependency bridge
                # the ~10us initial input-DMA wait and keep the HAM clock
                # gate at 8/8 so QKV starts at 2.4 GHz.
                for w in range(34):
                    pw = psx.tile([128, QT_], F32, tag="ps", name="ps")
                    wn = QT_ if w < 22 else 128
                    nc.tensor.matmul(pw[:, 0:wn], warm_sb[:, 0:128],
                                     warm_sb[:, 0:wn], start=True,
                                     stop=True)

                # wq on sync (HWDGE, contiguous) in 2-chunk pieces
                for kd0 in range(0, n_kd, 2):
                    nc.sync.dma_start(out=wq_sb[:, kd0:kd0 + 2, :],
                                      in_=wqkvT.ap()[:, kd0:kd0 + 2, :])
                nc.gpsimd.dma_start(out=id_sb[:], in_=ident[:, :])
                nc.gpsimd.dma_start(out=dm_sb[:], in_=dmask[:, :])
                nc.gpsimd.dma_start(out=wo_sb[:], in_=woT[:, :])

                def emit_group_dma(g, eng, step):
                    xg = xp.tile([128, n_kd, QT_], st, tag="xg",
                                 name=f"xg{g}")
                    for kd0 in range(0, n_kd, step):
                        eng.dma_start(out=xg[:, kd0:kd0 + step, :],
                                      in_=xT.ap()[g, :, kd0:kd0 + step, :])
                    return xg

                def emit_qkv_drain(ps, g, m):
                    gsl = slice(g * QT_, (g + 1) * QT_)
                    if m == 0:
                        nc.vector.tensor_scalar_mul(
                            qT_sb[:, gsl], ps[:], scale_q)
                    elif m == 1:
                        nc.vector.tensor_copy(kT_sb[:, gsl], ps[:])
                    else:
                        nc.vector.tensor_copy(vT_tmp[:, gsl], ps[:])

                def emit_vtr_drain(ci, pst):
                    for h in range(h_loc):
                        nc.vector.tensor_copy(
                            va_sb[h][:, ci, 0:dk],
                            pst[:, h * dk:(h + 1) * dk])
                        if av_fp8:
                            nc.vector.tensor_copy(
                                va8_sb[h][:, ci, 0:dk],
                                pst[:, h * dk:(h + 1) * dk])

                xg_map = {}
                filler = deque()
                boundary = deque()
                qkv_pending = {}

                def fill(k):
                    for _ in range(k):
                        if not filler:
                            return
                        filler.popleft()()

                def drain_boundary():
                    while boundary:
                        boundary.popleft()()

                def fill_until_qkv_done(g):
                    # correctness guard: attention on batch 1 must not be
                    # emitted before the filler QKV steps producing its
                    # qT/kT/va have been emitted (engine queues are FIFO; a
                    # consumer emitted before its producer on the same
                    # queue deadlocks).  Called at tile boundaries only, so
                    # draining deferred transposes here is safe.
                    while qkv_pending.get(g, 0) > 0:
                        if filler:
                            filler.popleft()()
                        else:
                            assert boundary, "filler underflow"
                            drain_boundary()

                def push_qkv_group(g):
                    # filler steps for one QKV group.  Each step is an
                    # ATOMIC accumulation chain (opens and closes its PSUM
                    # group within the step); PE transposes are deferred to
                    # tile boundaries where no po group is open.
                    cell = {}
                    xg = xg_map[g]

                    def s_mm(m):
                        def go():
                            cell[m] = psx.tile([128, QT_], F32,
                                               tag="ps", name="ps")
                            for kd in range(n_kd):
                                nc.tensor.matmul(
                                    cell[m][:],
                                    wq_sb[:, kd, m * 128:(m + 1) * 128],
                                    xg[:, kd, :],
                                    start=(kd == 0),
                                    stop=(kd == n_kd - 1),
                                    skip_group_check=True,
                                )
                            qkv_pending[g] -= 1
                        return go

                    def s_drain(m):
                        def go():
                            emit_qkv_drain(cell.pop(m), g, m)
                            qkv_pending[g] -= 1
                        return go

                    nsteps = 0
                    for m in range(3):
                        filler.append(s_mm(m))
                        filler.append(s_drain(m))
                        nsteps += 2
                    for ci in range(g * (QT_ // 128),
                                    (g + 1) * (QT_ // 128)):
                        def s_tr(ci=ci, g=g):
                            def tr():
                                pst = psx.tile([128, 128], vtdt,
                                               tag="ps", name="ps")
                                nc.tensor.transpose(
                                    pst[:],
                                    vT_tmp[:, ci * 128:(ci + 1) * 128],
                                    id_sb[:])
                                emit_vtr_drain(ci, pst)
                                qkv_pending[g] -= 1
                            boundary.append(tr)
                        filler.append(s_tr)
                        nsteps += 1
                    qkv_pending[g] = nsteps

                def emit_odma(tg, ob, on_scalar=False):
                    eng = nc.scalar if on_scalar else nc.sync
                    eng.dma_start(out=part[tg:tg + 128, :], in_=ob[:])

                ob_cell = {}

                def push_outproj(bi_, qt_):
                    # two filler steps per 128-token chunk (one per
                    # 512-wide half) to keep filler granularity fine
                    def s_half(tt, no):
                        def go():
                            tg = bi_ * s + tt * 128
                            if no == 0:
                                ob_cell[0] = obp.tile(
                                    [128, d], odt, tag="ob", name="ob")
                            ob = ob_cell[0]
                            px = psx.tile([128, QT_], F32, tag="ps",
                                          name="px")
                            nc.tensor.matmul(
                                px[:],
                                oT_sb[:, tg:tg + 128],
                                wo_sb[:, no * 512:(no + 1) * 512],
                                start=True, stop=True,
                            )
                            # drains stay on DVE mid-run: a blocked
                            # scalar.copy would head-of-line block the
                            # exp stream in the ACT FIFO queue
                            nc.vector.tensor_copy(
                                ob[:, no * 512:(no + 1) * 512], px[:])
                            if no == d // 512 - 1:
                                emit_odma(tg, ob_cell.pop(0))
                        return go
                    for tt in range(4 * qt_, 4 * qt_ + 4):
                        for no in range(d // 512):
                            filler.append(s_half(tt, no))

                def emit_attention_tile(bi, qt, last=False, fill_n=1):
                    q0 = bi * s + qt * QT_
                    n_full = qt * (QT_ // KC)
                    n_chunks = n_full + (QT_ // KC)
                    po = [pso.tile([128, QT_], F32, tag=f"po{h}",
                                   name=f"po{h}") for h in range(h_loc)]
                    base = bi * (s // KC)

                    def emit_av(h, pt, qoffs, full_pair, pair0):
                        if full_pair:
                            nc.tensor.matmul(
                                po[h][:],
                                va8_sb[h][:, base + pair0:
                                          base + pair0 + 2, :],
                                pt[:, :, :],
                                start=(pair0 == 0),
                                stop=False,
                                perf_mode=mybir.MatmulPerfMode.DoubleRow,
                                skip_group_check=True,
                            )
                            return
                        for jj in range(2):
                            kc = pair0 + jj
                            qoff = qoffs[jj]
                            if kc >= n_full:
                                nc.vector.tensor_mul(
                                    pt[:, jj, qoff:qoff + 128],
                                    pt[:, jj, qoff:qoff + 128],
                                    dm_sb[:])
                            nc.tensor.matmul(
                                po[h][:, qoff:QT_],
                                va_sb[h][:, base + kc, :],
                                pt[:, jj, qoff:QT_],
                                start=(kc == 0),
                                stop=(kc == n_chunks - 1),
                                skip_group_check=True,
                            )

                    for pair0 in range(0, n_chunks, 2):
                        for h in range(h_loc):
                            hp = slice(h * dk, (h + 1) * dk)
                            ps = pss.tile([128, 2, QT_], F32, tag="ps")
                            qoffs = []
                            # masked chunk j+1's scores stream starts at
                            # the pair base so ONE exp covers the pair
                            qe = (128 * (pair0 - n_full)
                                  if pair0 >= n_full else 0)
                            for jj in range(2):
                                kc = pair0 + jj
                                masked = kc >= n_full
                                j = kc - n_full if masked else 0
                                qoff = 128 * j if masked else 0
                                qoffs.append(qoff)
                                k0 = bi * s + kc * KC
                                nc.tensor.matmul(
                                    ps[:, jj, qe:QT_],
                                    kT_sb[hp, k0:k0 + KC],
                                    qT_sb[hp, q0 + qe:q0 + QT_],
                                    start=True, stop=True,
                                    skip_group_check=True,
                                )
                            full_pair = av_fp8 and pair0 + 1 < n_full
                            pt = ptp.tile([128, 2, QT_],
                                          avdt if full_pair else st,
                                          tag="pt8" if full_pair
                                          else "pt")
                            nc.scalar.activation(
                                pt[:, :, qe:QT_],
                                ps[:, :, qe:QT_],
                                mybir.ActivationFunctionType.Exp,
                                bias=ebias_c[:, 0:1])
                            emit_av(h, pt, qoffs, full_pair, pair0)
                            fill(fill_n)
                    if last:
                        return po
                    # normalization (HW-proven pattern): stage po halves to
                    # partition-base-0 SBUF tiles, reciprocal from base 0,
                    # one multiply into oT.
                    for h in range(h_loc):
                        hp_ = slice(h * dk, (h + 1) * dk)
                        o_n = onp.tile([dk, QT_], F32, tag=f"onn{h}",
                                       name=f"onn{h}")
                        nc.vector.tensor_copy(o_n[:], po[h][0:dk, :])
                        o_d = onp.tile([dk, QT_], F32, tag=f"ond{h}",
                                       name=f"ond{h}")
                        nc.vector.tensor_copy(o_d[:], po[h][dk:2 * dk, :])
                        rb = onp.tile([dk, QT_], F32, tag=f"rb{h}",
                                      name=f"rb{h}")
                        nc.vector.reciprocal_approx_fast(rb[:], o_d[:])
                        nc.vector.tensor_mul(
                            oT_sb[hp_, q0:q0 + QT_], o_n[:], rb[:])
                    return None

                # phase A: batch-0 QKV projection, executed eagerly
                # through the same step machinery as the filler path.
                for g in range(n_groups // 2):
                    eng = nc.scalar if g % 2 == 0 else nc.sync
                    xg_map[g] = emit_group_dma(g, eng, 2 if g < 2 else 4)
                    push_qkv_group(g)
                    fill_until_qkv_done(g)

                # prefetch all batch-1 x groups now on the same queues
                # (FIFO keeps them behind batch 0's input stream); the xg
                # pool is 8-deep so there are no slot-reuse waits and
                # filler matmuls never stall the PE on DMA.
                for g in range(n_groups // 2, n_groups):
                    eng = nc.scalar if g % 2 == 0 else nc.sync
                    xg_map[g] = emit_group_dma(g, eng, 2)

                # phase B: batch-0 attention; push batch-1 QKV groups one
                # tile late so their input DMA has landed.
                for qt in range(n_qt):
                    emit_attention_tile(0, qt)
                    drain_boundary()
                    push_qkv_group(n_groups // 2 + qt)
                    if qt >= 2:
                        push_outproj(0, qt - 2)
                # phase C: batch-1 attention; filler is the remaining QKV
                # steps then the deferred output projections, pushed late
                # so the exp-paced final tiles still have PE filler.
                push_outproj(0, n_qt - 2)
                po_last = None
                for qt in range(n_qt):
                    if qt == 1:
                        push_outproj(0, n_qt - 1)
                    if qt >= 1:
                        push_outproj(1, qt - 1)
                    fill_until_qkv_done(n_groups // 2 + qt)
                    last = qt == n_qt - 1
                    po_last = emit_attention_tile(1, qt, last=last)
                fill(len(filler))

                # tail: the last tile normalizes per 128-token slab so its
                # o_proj matmuls start before the whole tile is done;
                # drains alternate DVE/ACT; half-tile DMAs on both queues.
                bi_, qt_ = 1, n_qt - 1
                q0_ = bi_ * s + qt_ * QT_
                stg = []
                for h in range(h_loc):
                    o_n = onp.tile([dk, QT_], F32, tag=f"onn{h}",
                                   name=f"onn{h}")
                    nc.vector.tensor_copy(o_n[:], po_last[h][0:dk, :])
                    o_d = onp.tile([dk, QT_], F32, tag=f"ond{h}",
                                   name=f"ond{h}")
                    nc.vector.tensor_copy(o_d[:], po_last[h][dk:2 * dk, :])
                    rb = onp.tile([dk, QT_], F32, tag=f"rb{h}",
                                  name=f"rb{h}")
                    nc.vector.reciprocal_approx_fast(rb[:], o_d[:])
                    stg.append((o_n, rb))
                for ti in range(4):
                    sl = slice(ti * 128, (ti + 1) * 128)
                    for h in range(h_loc):
                        hp_ = slice(h * dk, (h + 1) * dk)
                        nc.vector.tensor_mul(
                            oT_sb[hp_,
                                  q0_ + ti * 128:q0_ + (ti + 1) * 128],
                            stg[h][0][:, sl], stg[h][1][:, sl])
                    tt = 4 * qt_ + ti
                    tg = bi_ * s + tt * 128
                    ob = obp.tile([128, d], odt, tag="ob", name="ob")
                    for no in range(d // 512):
                        px = psx.tile([128, QT_], F32, tag="ps",
                                      name="px")
                        nc.tensor.matmul(
                            px[:],
                            oT_sb[:, tg:tg + 128],
                            wo_sb[:, no * 512:(no + 1) * 512],
                            start=True, stop=True,
                        )
                        if ti % 2:
                            nc.scalar.copy(
                                ob[:, no * 512:(no + 1) * 512], px[:])
                        else:
                            nc.vector.tensor_copy(
                                ob[:, no * 512:(no + 1) * 512], px[:])
                        eng = nc.scalar if (ti + no) % 2 else nc.sync
                        eng.dma_start(
                            out=part[tg:tg + 128,
                                     no * 512:(no + 1) * 512],
                            in_=ob[:, no * 512:(no + 1) * 512])

                if debug_dump:
                    nc.sync.dma_start(out=dbg["qT"].ap(), in_=qT_sb[:])
                    nc.sync.dma_start(out=dbg["kT"].ap(), in_=kT_sb[:])
                    nc.sync.dma_start(out=dbg["va0"].ap(), in_=va_sb[0][:])
                    nc.sync.dma_start(out=dbg["oT"].ap(), in_=oT_sb[:])
                    nc.sync.dma_start(out=dbg["wo"].ap(), in_=wo_sb[:])

    nc.compile()
    return nc


_NC_CACHE = {}


def _get_program(mm_mode=None):
    if mm_mode is None:
        mm_mode = MM_MODE
    if mm_mode not in _NC_CACHE:
        _NC_CACHE[mm_mode] = build_program(mm_mode)
    return _NC_CACHE[mm_mode]


def make_host_inputs(in_features, qkv_proj_weight, o_proj_weight,
                     mm_mode=None, ncores=NCORES):
    """Build the per-core input maps (the TP shard step)."""
    x = np.asarray(in_features, dtype=np.float32)
    qkv = np.asarray(qkv_proj_weight, dtype=np.float32)
    wo = np.asarray(o_proj_weight, dtype=np.float32)
    b, s, d = x.shape
    if mm_mode is None:
        mm_mode = MM_MODE
    np_st = ml_dtypes.bfloat16 if mm_mode == "bf16" else np.float32
    np_vt = ml_dtypes.bfloat16 if mm_mode == "bf16" else np.float32

    # pre-swizzled fully-contiguous DMA layouts (see build_program):
    # xT[g, p, c, n] = x[g*512+n, c*128+p]
    qt_, t_all = 512, b * s
    n_groups, n_kd = t_all // qt_, d // 128
    xTg = np.ascontiguousarray(
        x.reshape(n_groups, qt_, n_kd, 128).transpose(0, 3, 2, 1)
    ).astype(np_st)
    kk = np.arange(128)[:, None]
    qq = np.arange(128)[None, :]
    dmask = (qq >= kk).astype(np_st)
    ident = np.eye(128, dtype=np_vt)

    in_maps = []
    for c in range(ncores):
        rows = slice(c * DLOC, (c + 1) * DLOC)
        wstack = np.concatenate([qkv[i][rows, :] for i in range(3)], axis=0)
        # wqkvT[p, c, n] = wstack[n, c*128+p]
        wqkvT = np.ascontiguousarray(
            wstack.T.reshape(n_kd, 128, 3 * DLOC).transpose(1, 0, 2)
        ).astype(np_st)
        woT = np.ascontiguousarray(wo[:, rows].T).astype(np_st)
        in_maps.append({"xT": xTg, "wqkvT": wqkvT, "woT": woT,
                        "dmask": dmask, "ident": ident})
    return in_maps


def kernel(**inputs):
    nh = inputs.get("num_heads", NH)
    nh = int(np.asarray(nh)) if not isinstance(nh, int) else nh
    assert nh == NH, f"kernel hardcoded for {NH} heads, got {nh}"

    nc = _get_program()
    in_maps = make_host_inputs(inputs["in_features"],
                               inputs["qkv_proj_weight"],
                               inputs["o_proj_weight"])
    res = run_bass_kernel_spmd(nc, in_maps, list(range(NCORES)))
    acc = np.zeros((T, D), dtype=np.float64)
    for c in range(NCORES):
        acc += np.asarray(res.results[c]["part"], dtype=np.float64)
    return acc.reshape(B, S, D).astype(np.float32)


# revision 38
# speedup vs baseline: 1.0134x; 1.0134x over previous
"""Causal multi-head self-attention (B=2, S=2048, D=1024, H=16) on 8 trn2 cores.

Sharding: tensor-parallel over heads. Each core owns 2 heads (a 128-wide
slice of the QKV output dim / o_proj input dim), computes QKV projection,
causal attention and its partial output projection; the host sums the 8
partial outputs (the TP unshard step).

Device kernel layout (per core, SPMD):
  - Everything is computed in "transposed" orientation so no transposes of
    attention probabilities are ever needed:
      qT, kT  : [dloc=128, T]   (feature-major)  qT pre-scaled by 1/sqrt(dk)
      scoresT : [k_chunk=128, q_tile=512] = kT_chunk.T-contract @ qT
      PT      : exp(scoresT)  (block-causal; diagonal 128x128 blocks masked
                by a DVE multiply with an uploaded 0/1 mask)
      oT      : accum over k chunks of V_aug.T @ PT, where V_aug carries a
                64-wide ones block so PSUM rows 64:128 hold 64 replicated
                copies of the softmax denominator (free PE broadcast).
      out     : [tok, 1024] partial = oT(normalized) as lhsT @ woT
  - Unmasked chunk pairs run the AV matmul as one fp8e4m3 DoubleRow pass
    (2 k-chunks per 512-column stream); masked (diagonal) chunks stay bf16
    so short causal rows keep bf16 accuracy.
  - Software pipeline: batch 0's QKV projection runs first (PE-bound, ACT
    idle); batch 0's attention (ACT/exp-paced) then runs with batch 1's
    QKV chains interleaved as PE filler between attention pair-blocks;
    batch 1's attention runs with the deferred output projections as
    filler.  This keeps the PE issue queue non-blocking through the
    exp-paced stretches instead of phase-serial QKV-then-attention.
  - One PSUM pool set for the whole kernel: pss 2x[128,2,512] score pairs
    (4 banks), pso 2x[128,512] AV accumulators (2 banks), psx 2x[128,512]
    shared ring for warmup/QKV chains/V transposes/o_proj (2 banks).
  - Normalization: po staged to partition-base-0 SBUF tiles, then
    reciprocal_approx_fast + one multiply into oT.  NOTE (HW-proven):
    reciprocal_approx_fast must read a base-0 tile -- feeding it a
    partition-offset slice (po[64:128] or an SBUF slice at offset 64)
    produces garbage on hardware while CoreSim accepts it.

dtype modes: "bf16" (default/fastest, rel err ~5e-3 vs the 2e-2 gate),
"f32r" (tf32-ish matmuls, ~2e-4), "f32" (exact, PE at 1/4 rate).
"""

import numpy as np
import ml_dtypes
from collections import deque

import concourse.bass as bass
import concourse.mybir as mybir
import concourse.tile as tile
from concourse import bacc
from concourse.bass_utils import run_bass_kernel_spmd

# Problem config (hardcoded; harness contract).
B, S, D, NH = 2, 2048, 1024, 16
NCORES = 8
DK = D // NH                # 64
H_LOC = NH // NCORES        # 2 heads per core
DLOC = H_LOC * DK           # 128
T = B * S                   # 4096

MM_MODE = "bf16"            # "f32" | "f32r" | "bf16"

F32 = mybir.dt.float32
BF16 = mybir.dt.bfloat16
F32R = mybir.dt.float32r


def build_program(mm_mode=MM_MODE, b=B, s=S, d=D, nh=NH, ncores=NCORES,
                  debug_dump=False):
    dk = d // nh
    h_loc = nh // ncores
    dloc = h_loc * dk
    t_all = b * s

    QT_ = 512                     # q tile size (matmul moving free dim)
    KC = 128                      # k chunk size (partition dim)
    assert s % QT_ == 0 and d % 128 == 0 and dloc == 128 and dk == 64

    n_qt = s // QT_               # q tiles per batch
    n_groups = t_all // QT_       # token groups for QKV projection
    n_kd = d // 128               # contraction chunks over d_model
    n_tc = t_all // KC            # token chunks (for V layout)

    st = {"bf16": BF16, "f32r": F32R, "f32": F32}[mm_mode]
    avdt = mybir.dt.float8e4 if mm_mode == "bf16" else st
    av_fp8 = avdt == mybir.dt.float8e4
    exp_bias = -1.0 if av_fp8 else 0.0
    vtdt = BF16 if mm_mode == "bf16" else F32
    odt = BF16 if mm_mode == "bf16" else F32

    nc = bacc.Bacc("TRN2", target_bir_lowering=False, debug=False,
                   enable_asserts=False)

    # xT / wqkvT are host-pre-swizzled into the exact SBUF tile layouts so
    # every DMA is fully contiguous (strided patterns measured ~4x slower)
    xT = nc.dram_tensor("xT", [n_groups, 128, n_kd, QT_], st,
                        kind="ExternalInput")
    wqkvT = nc.dram_tensor("wqkvT", [128, n_kd, 3 * dloc], st,
                           kind="ExternalInput")
    woT = nc.dram_tensor("woT", [dloc, d], st, kind="ExternalInput")
    dmask = nc.dram_tensor("dmask", [128, 128], st, kind="ExternalInput")
    ident = nc.dram_tensor("ident", [128, 128], vtdt, kind="ExternalInput")
    part = nc.dram_tensor("part", [t_all, d], odt, kind="ExternalOutput")
    dbg = {}
    if debug_dump:
        for nm, shp, dt_ in [("qT", [128, t_all], st),
                             ("kT", [128, t_all], st),
                             ("va0", [128, t_all // 128, 128], st),
                             ("oT", [128, t_all], st),
                             ("wo", [dloc, d], st)]:
            dbg[nm] = nc.dram_tensor(f"dbg_{nm}", shp, dt_,
                                     kind="ExternalOutput")

    with tile.TileContext(nc) as tc:
        with (
            tc.tile_pool(name="persist", bufs=1) as pp,
            tc.tile_pool(name="small", bufs=4) as sp,
        ):
            # ---- persistent SBUF tensors ----
            wo_sb = pp.tile([dloc, d], st, tag="wo")
            dm_sb = pp.tile([128, 128], st, tag="dm")
            id_sb = pp.tile([128, 128], vtdt, tag="id")

            # scratch for PE warm-up matmuls (HAM clock-gate release)
            warm_sb = pp.tile([128, 512], st, tag="warm")
            nc.vector.memset(warm_sb[:], 0.0)

            qT_sb = pp.tile([128, t_all], st, tag="qT")
            kT_sb = pp.tile([128, t_all], st, tag="kT")
            va_sb = [pp.tile([128, n_tc, 128], st, tag=f"va{h}",
                             name=f"va{h}") for h in range(h_loc)]
            va8_sb = [pp.tile([128, n_tc, 128], avdt, tag=f"va8{h}",
                              name=f"va8{h}") for h in range(h_loc)
                      ] if av_fp8 else None
            ones_c = pp.tile([128, 1], F32, tag="ones_c")
            nc.vector.memset(ones_c[:], 1.0)
            ebias_c = pp.tile([128, 1], F32, tag="ebias_c")
            nc.vector.memset(ebias_c[:], exp_bias)
            for h in range(h_loc):
                nc.vector.tensor_copy(
                    va_sb[h][:, :, dk:128],
                    ones_c[:, :, None].broadcast_to((128, n_tc, 128 - dk)))
                if av_fp8:
                    nc.vector.tensor_copy(
                        va8_sb[h][:, :, dk:128],
                        ones_c[:, :, None].broadcast_to(
                            (128, n_tc, 128 - dk)))
            oT_sb = pp.tile([128, t_all], st, tag="oT")

            scale_q = 1.0 / float(np.sqrt(dk))

            with (
                tc.tile_pool(name="qkvw", bufs=1) as wp,
                tc.tile_pool(name="xg", bufs=8) as xp,
                tc.tile_pool(name="vtmp", bufs=1) as vp,
                tc.tile_pool(name="pt", bufs=10) as ptp,
                tc.tile_pool(name="onb", bufs=2) as onp,
                tc.tile_pool(name="ob", bufs=4) as obp,
                tc.tile_pool(name="ps_s", bufs=2, space="PSUM") as pss,
                tc.tile_pool(name="ps_o", bufs=1, space="PSUM") as pso,
                tc.tile_pool(name="ps_x", bufs=2, space="PSUM") as psx,
            ):
                wq_sb = wp.tile([128, n_kd, 3 * dloc], st, tag="wq")
                vT_tmp = vp.tile([128, t_all], vtdt, tag="vtmp")

                # PE warm-up: dummy matmuls with no DMA dependency bridge
                # the ~10us initial input-DMA wait and keep the HAM clock
                # gate at 8/8 so QKV starts at 2.4 GHz.
                for w in range(34):
                    pw = psx.tile([128, QT_], F32, tag="ps", name="ps")
                    wn = QT_ if w < 22 else 128
                    nc.tensor.matmul(pw[:, 0:wn], warm_sb[:, 0:128],
                                     warm_sb[:, 0:wn], start=True,
                                     stop=True)

                # wq on sync (HWDGE, contiguous) in 2-chunk pieces
                for kd0 in range(0, n_kd, 2):
                    nc.sync.dma_start(out=wq_sb[:, kd0:kd0 + 2, :],
                                      in_=wqkvT.ap()[:, kd0:kd0 + 2, :])
                nc.gpsimd.dma_start(out=id_sb[:], in_=ident[:, :])
                nc.gpsimd.dma_start(out=dm_sb[:], in_=dmask[:, :])
                nc.gpsimd.dma_start(out=wo_sb[:], in_=woT[:, :])

                def emit_group_dma(g, eng, step):
                    xg = xp.tile([128, n_kd, QT_], st, tag="xg",
                                 name=f"xg{g}")
                    for kd0 in range(0, n_kd, step):
                        eng.dma_start(out=xg[:, kd0:kd0 + step, :],
                                      in_=xT.ap()[g, :, kd0:kd0 + step, :])
                    return xg

                def emit_qkv_drain(ps, g, m, use_act=False):
                    # K/V drains can ride the ACT engine (53% idle): they
                    # wait only on the just-finished chain matmul, so no
                    # head-of-line risk for the exp stream.
                    gsl = slice(g * QT_, (g + 1) * QT_)
                    if m == 0:
                        nc.vector.tensor_scalar_mul(
                            qT_sb[:, gsl], ps[:], scale_q)
                    elif m == 1:
                        (nc.scalar.copy if use_act
                         else nc.vector.tensor_copy)(kT_sb[:, gsl], ps[:])
                    else:
                        (nc.scalar.copy if use_act
                         else nc.vector.tensor_copy)(vT_tmp[:, gsl], ps[:])

                def emit_vtr_drain(ci, pst):
                    for h in range(h_loc):
                        nc.vector.tensor_copy(
                            va_sb[h][:, ci, 0:dk],
                            pst[:, h * dk:(h + 1) * dk])
                        if av_fp8:
                            nc.vector.tensor_copy(
                                va8_sb[h][:, ci, 0:dk],
                                pst[:, h * dk:(h + 1) * dk])

                xg_map = {}
                filler = deque()
                boundary = deque()
                qkv_pending = {}

                def fill(k):
                    for _ in range(k):
                        if not filler:
                            return
                        filler.popleft()()

                def drain_boundary():
                    while boundary:
                        boundary.popleft()()

                def fill_until_qkv_done(g):
                    # correctness guard: attention on batch 1 must not be
                    # emitted before the filler QKV steps producing its
                    # qT/kT/va have been emitted (engine queues are FIFO; a
                    # consumer emitted before its producer on the same
                    # queue deadlocks).  Called at tile boundaries only, so
                    # draining deferred transposes here is safe.
                    while qkv_pending.get(g, 0) > 0:
                        if filler:
                            filler.popleft()()
                        else:
                            assert boundary, "filler underflow"
                            drain_boundary()

                def push_qkv_group(g):
                    # filler steps for one QKV group.  Each step is an
                    # ATOMIC accumulation chain (opens and closes its PSUM
                    # group within the step); PE transposes are deferred to
                    # tile boundaries where no po group is open.
                    cell = {}
                    xg = xg_map[g]

                    def s_mm(m):
                        def go():
                            cell[m] = psx.tile([128, QT_], F32,
                                               tag="ps", name="ps")
                            for kd in range(n_kd):
                                nc.tensor.matmul(
                                    cell[m][:],
                                    wq_sb[:, kd, m * 128:(m + 1) * 128],
                                    xg[:, kd, :],
                                    start=(kd == 0),
                                    stop=(kd == n_kd - 1),
                                    skip_group_check=True,
                                )
                            qkv_pending[g] -= 1
                        return go

                    def s_drain(m):
                        def go():
                            emit_qkv_drain(cell.pop(m), g, m,
                                           use_act=(g >= n_groups // 2))
                            qkv_pending[g] -= 1
                        return go

                    nsteps = 0
                    for m in range(3):
                        filler.append(s_mm(m))
                        filler.append(s_drain(m))
                        nsteps += 2
                    for ci in range(g * (QT_ // 128),
                                    (g + 1) * (QT_ // 128)):
                        def s_tr(ci=ci, g=g):
                            def tr():
                                pst = psx.tile([128, 128], vtdt,
                                               tag="ps", name="ps")
                                nc.tensor.transpose(
                                    pst[:],
                                    vT_tmp[:, ci * 128:(ci + 1) * 128],
                                    id_sb[:])
                                emit_vtr_drain(ci, pst)
                                qkv_pending[g] -= 1
                            boundary.append(tr)
                        filler.append(s_tr)
                        nsteps += 1
                    qkv_pending[g] = nsteps

                def emit_odma(tg, ob, on_scalar=False):
                    eng = nc.scalar if on_scalar else nc.sync
                    eng.dma_start(out=part[tg:tg + 128, :], in_=ob[:])

                ob_cell = {}

                def push_outproj(bi_, qt_):
                    # two filler steps per 128-token chunk (one per
                    # 512-wide half) to keep filler granularity fine
                    def s_half(tt, no):
                        def go():
                            tg = bi_ * s + tt * 128
                            if no == 0:
                                ob_cell[0] = obp.tile(
                                    [128, d], odt, tag="ob", name="ob")
                            ob = ob_cell[0]
                            px = psx.tile([128, QT_], F32, tag="ps",
                                          name="px")
                            nc.tensor.matmul(
                                px[:],
                                oT_sb[:, tg:tg + 128],
                                wo_sb[:, no * 512:(no + 1) * 512],
                                start=True, stop=True,
                            )
                            # drains stay on DVE mid-run: a blocked
                            # scalar.copy would head-of-line block the
                            # exp stream in the ACT FIFO queue
                            nc.vector.tensor_copy(
                                ob[:, no * 512:(no + 1) * 512], px[:])
                            if no == d // 512 - 1:
                                emit_odma(tg, ob_cell.pop(0))
                        return go
                    for tt in range(4 * qt_, 4 * qt_ + 4):
                        for no in range(d // 512):
                            filler.append(s_half(tt, no))

                def emit_attention_tile(bi, qt, last=False, fill_n=1):
                    q0 = bi * s + qt * QT_
                    n_full = qt * (QT_ // KC)
                    n_chunks = n_full + (QT_ // KC)
                    po = [pso.tile([128, QT_], F32, tag=f"po{h}",
                                   name=f"po{h}") for h in range(h_loc)]
                    base = bi * (s // KC)

                    def emit_av(h, pt, qoffs, full_pair, pair0):
                        if full_pair:
                            nc.tensor.matmul(
                                po[h][:],
                                va8_sb[h][:, base + pair0:
                                          base + pair0 + 2, :],
                                pt[:, :, :],
                                start=(pair0 == 0),
                                stop=False,
                                perf_mode=mybir.MatmulPerfMode.DoubleRow,
                                skip_group_check=True,
                            )
                            return
                        for jj in range(2):
                            kc = pair0 + jj
                            qoff = qoffs[jj]
                            if kc >= n_full:
                                nc.vector.tensor_mul(
                                    pt[:, jj, qoff:qoff + 128],
                                    pt[:, jj, qoff:qoff + 128],
                                    dm_sb[:])
                            nc.tensor.matmul(
                                po[h][:, qoff:QT_],
                                va_sb[h][:, base + kc, :],
                                pt[:, jj, qoff:QT_],
                                start=(kc == 0),
                                stop=(kc == n_chunks - 1),
                                skip_group_check=True,
                            )

                    for pair0 in range(0, n_chunks, 2):
                        for h in range(h_loc):
                            hp = slice(h * dk, (h + 1) * dk)
                            ps = pss.tile([128, 2, QT_], F32, tag="ps")
                            qoffs = []
                            # masked chunk j+1's scores stream starts at
                            # the pair base so ONE exp covers the pair
                            qe = (128 * (pair0 - n_full)
                                  if pair0 >= n_full else 0)
                            for jj in range(2):
                                kc = pair0 + jj
                                masked = kc >= n_full
                                j = kc - n_full if masked else 0
                                qoff = 128 * j if masked else 0
                                qoffs.append(qoff)
                                k0 = bi * s + kc * KC
                                nc.tensor.matmul(
                                    ps[:, jj, qe:QT_],
                                    kT_sb[hp, k0:k0 + KC],
                                    qT_sb[hp, q0 + qe:q0 + QT_],
                                    start=True, stop=True,
                                    skip_group_check=True,
                                )
                            full_pair = av_fp8 and pair0 + 1 < n_full
                            pt = ptp.tile([128, 2, QT_],
                                          avdt if full_pair else st,
                                          tag="pt8" if full_pair
                                          else "pt")
                            nc.scalar.activation(
                                pt[:, :, qe:QT_],
                                ps[:, :, qe:QT_],
                                mybir.ActivationFunctionType.Exp,
                                bias=ebias_c[:, 0:1])
                            emit_av(h, pt, qoffs, full_pair, pair0)
                            fill(fill_n)
                    if last:
                        return po
                    # normalization (HW-proven pattern): stage po halves to
                    # partition-base-0 SBUF tiles, reciprocal from base 0,
                    # one multiply into oT.
                    for h in range(h_loc):
                        hp_ = slice(h * dk, (h + 1) * dk)
                        o_n = onp.tile([dk, QT_], F32, tag=f"onn{h}",
                                       name=f"onn{h}")
                        nc.vector.tensor_copy(o_n[:], po[h][0:dk, :])
                        o_d = onp.tile([dk, QT_], F32, tag=f"ond{h}",
                                       name=f"ond{h}")
                        nc.vector.tensor_copy(o_d[:], po[h][dk:2 * dk, :])
                        rb = onp.tile([dk, QT_], F32, tag=f"rb{h}",
                                      name=f"rb{h}")
                        nc.vector.reciprocal_approx_fast(rb[:], o_d[:])
                        nc.vector.tensor_mul(
                            oT_sb[hp_, q0:q0 + QT_], o_n[:], rb[:])
                    return None

                # phase A: batch-0 QKV projection, executed eagerly
                # through the same step machinery as the filler path.
                for g in range(n_groups // 2):
                    eng = nc.scalar if g % 2 == 0 else nc.sync
                    xg_map[g] = emit_group_dma(g, eng, 2 if g < 2 else 4)
                    push_qkv_group(g)
                    fill_until_qkv_done(g)

                # batch-1 x prefetch: first group right away (needed one
                # b0-tile later), the rest spread one per tile boundary on
                # the sync queue so they never contend with batch 0's
                # input burst for HBM.  The xg pool is 8-deep so there are
                # no slot-reuse waits and filler matmuls never stall the
                # PE on DMA.
                xg_map[n_groups // 2] = emit_group_dma(
                    n_groups // 2, nc.sync, 4)

                # phase B: batch-0 attention; push batch-1 QKV groups one
                # tile late so their input DMA has landed.
                for qt in range(n_qt):
                    emit_attention_tile(0, qt)
                    drain_boundary()
                    g_next = n_groups // 2 + qt + 1
                    if g_next < n_groups:
                        xg_map[g_next] = emit_group_dma(g_next, nc.sync, 4)
                    push_qkv_group(n_groups // 2 + qt)
                    if qt >= 2:
                        push_outproj(0, qt - 2)
                # phase C: batch-1 attention; filler is the remaining QKV
                # steps then the deferred output projections.
                push_outproj(0, n_qt - 2)
                push_outproj(0, n_qt - 1)
                po_last = None
                for qt in range(n_qt):
                    if qt >= 1:
                        push_outproj(1, qt - 1)
                    fill_until_qkv_done(n_groups // 2 + qt)
                    last = qt == n_qt - 1
                    po_last = emit_attention_tile(1, qt, last=last)
                fill(len(filler))

                # tail: the last tile normalizes per 128-token slab so its
                # o_proj matmuls start before the whole tile is done;
                # drains alternate DVE/ACT; half-tile DMAs on both queues.
                bi_, qt_ = 1, n_qt - 1
                q0_ = bi_ * s + qt_ * QT_
                stg = []
                for h in range(h_loc):
                    o_n = onp.tile([dk, QT_], F32, tag=f"onn{h}",
                                   name=f"onn{h}")
                    nc.vector.tensor_copy(o_n[:], po_last[h][0:dk, :])
                    o_d = onp.tile([dk, QT_], F32, tag=f"ond{h}",
                                   name=f"ond{h}")
                    nc.vector.tensor_copy(o_d[:], po_last[h][dk:2 * dk, :])
                    rb = onp.tile([dk, QT_], F32, tag=f"rb{h}",
                                  name=f"rb{h}")
                    nc.vector.reciprocal_approx_fast(rb[:], o_d[:])
                    stg.append((o_n, rb))
                for ti in range(4):
                    sl = slice(ti * 128, (ti + 1) * 128)
                    for h in range(h_loc):
                        hp_ = slice(h * dk, (h + 1) * dk)
                        nc.vector.tensor_mul(
                            oT_sb[hp_,
                                  q0_ + ti * 128:q0_ + (ti + 1) * 128],
                            stg[h][0][:, sl], stg[h][1][:, sl])
                    tt = 4 * qt_ + ti
                    tg = bi_ * s + tt * 128
                    ob = obp.tile([128, d], odt, tag="ob", name="ob")
                    for no in range(d // 512):
                        px = psx.tile([128, QT_], F32, tag="ps",
                                      name="px")
                        nc.tensor.matmul(
                            px[:],
                            oT_sb[:, tg:tg + 128],
                            wo_sb[:, no * 512:(no + 1) * 512],
                            start=True, stop=True,
                        )
                        if ti % 2:
                            nc.scalar.copy(
                                ob[:, no * 512:(no + 1) * 512], px[:])
                        else:
                            nc.vector.tensor_copy(
                                ob[:, no * 512:(no + 1) * 512], px[:])
                        eng = nc.scalar if (ti + no) % 2 else nc.sync
                        eng.dma_start(
                            out=part[tg:tg + 128,
                                     no * 512:(no + 1) * 512],
                            in_=ob[:, no * 512:(no + 1) * 512])

                if debug_dump:
                    nc.sync.dma_start(out=dbg["qT"].ap(), in_=qT_sb[:])
                    nc.sync.dma_start(out=dbg["kT"].ap(), in_=kT_sb[:])
                    nc.sync.dma_start(out=dbg["va0"].ap(), in_=va_sb[0][:])
                    nc.sync.dma_start(out=dbg["oT"].ap(), in_=oT_sb[:])
                    nc.sync.dma_start(out=dbg["wo"].ap(), in_=wo_sb[:])

    nc.compile()
    return nc


_NC_CACHE = {}


def _get_program(mm_mode=None):
    if mm_mode is None:
        mm_mode = MM_MODE
    if mm_mode not in _NC_CACHE:
        _NC_CACHE[mm_mode] = build_program(mm_mode)
    return _NC_CACHE[mm_mode]


def make_host_inputs(in_features, qkv_proj_weight, o_proj_weight,
                     mm_mode=None, ncores=NCORES):
    """Build the per-core input maps (the TP shard step)."""
    x = np.asarray(in_features, dtype=np.float32)
    qkv = np.asarray(qkv_proj_weight, dtype=np.float32)
    wo = np.asarray(o_proj_weight, dtype=np.float32)
    b, s, d = x.shape
    if mm_mode is None:
        mm_mode = MM_MODE
    np_st = ml_dtypes.bfloat16 if mm_mode == "bf16" else np.float32
    np_vt = ml_dtypes.bfloat16 if mm_mode == "bf16" else np.float32

    # pre-swizzled fully-contiguous DMA layouts (see build_program):
    # xT[g, p, c, n] = x[g*512+n, c*128+p]
    qt_, t_all = 512, b * s
    n_groups, n_kd = t_all // qt_, d // 128
    xTg = np.ascontiguousarray(
        x.reshape(n_groups, qt_, n_kd, 128).transpose(0, 3, 2, 1)
    ).astype(np_st)
    kk = np.arange(128)[:, None]
    qq = np.arange(128)[None, :]
    dmask = (qq >= kk).astype(np_st)
    ident = np.eye(128, dtype=np_vt)

    in_maps = []
    for c in range(ncores):
        rows = slice(c * DLOC, (c + 1) * DLOC)
        wstack = np.concatenate([qkv[i][rows, :] for i in range(3)], axis=0)
        # wqkvT[p, c, n] = wstack[n, c*128+p]
        wqkvT = np.ascontiguousarray(
            wstack.T.reshape(n_kd, 128, 3 * DLOC).transpose(1, 0, 2)
        ).astype(np_st)
        woT = np.ascontiguousarray(wo[:, rows].T).astype(np_st)
        in_maps.append({"xT": xTg, "wqkvT": wqkvT, "woT": woT,
                        "dmask": dmask, "ident": ident})
    return in_maps


def kernel(**inputs):
    nh = inputs.get("num_heads", NH)
    nh = int(np.asarray(nh)) if not isinstance(nh, int) else nh
    assert nh == NH, f"kernel hardcoded for {NH} heads, got {nh}"

    nc = _get_program()
    in_maps = make_host_inputs(inputs["in_features"],
                               inputs["qkv_proj_weight"],
                               inputs["o_proj_weight"])
    res = run_bass_kernel_spmd(nc, in_maps, list(range(NCORES)))
    acc = np.zeros((T, D), dtype=np.float64)
    for c in range(NCORES):
        acc += np.asarray(res.results[c]["part"], dtype=np.float64)
    return acc.reshape(B, S, D).astype(np.float32)
